# revision 57
# baseline (speedup 1.0000x reference)
"""Multi-head attention (B=2, S=2048, EMB=1024, H=16, hd=64) on 8 TRN2 cores.

Sharding: core c -> batch b = c//4, head-group g = c%4 (4 heads, 256 emb dims).
Per core (all matmuls bf16: full-rate 1 cyc/row streaming):
  A) Q^T = Wq_g @ x_b^T   [256, 2048]   (transposed layout, contraction on emb)
     K^T = Wk_g @ x_b^T   [256, 2048]
     V   = x_b @ Wv_g^T   [2048, 256]   (natural layout, +ones column per head)
  B) per head h: S^T[k,q] = K_h @ Q_h^T (16 k-tiles x [128, 2048] psum)
     P^T = exp(S^T/8): split between ACT (native Exp) and DVE (custom 2-op
     chain: deg-4 poly p~exp(s/64) then p^8) so the softmax isn't ACT-bound;
     U_aug[65, 2048] += [V_h|1].T @ P^T  (row 64 = softmax sums)
  C) r = 1/sums (DVE recip approx); broadcast r over 64 partitions via
     indicator matmul; O^T = U^T * r on GpSimd (written over the Q^T buffer)
  D) y = O @ Wo_g^T partial [2048, 1024]; host sums the 4 head-group partials.
"""
import numpy as np

import concourse.bass as bass
import concourse.tile as tile
from concourse import bacc, mybir
from concourse.bass_utils import run_bass_kernel_spmd

import os

F32 = mybir.dt.float32
F32R = mybir.dt.float32r
BF16 = mybir.dt.bfloat16
FP16 = mybir.dt.float16
# matmul dtype: bf16 (1 cyc/row, ~6e-3) | fp16 (2 cyc/row, ~1e-3) | f32r
MM_DT_NAME = os.environ.get("MM_DT", "bf16")
MM = {"f32r": F32R, "bf16": BF16, "fp16": FP16}[MM_DT_NAME]
IN_DT = {"f32r": F32, "bf16": BF16, "fp16": FP16}[MM_DT_NAME]
EXP = mybir.ActivationFunctionType.Exp
MULT = mybir.AluOpType.mult

EMB = 1024
S = 2048
B = 2
HG = 4           # heads per core
HD = 64
CHD = HG * HD    # 256 emb dims per core
ET = EMB // 128  # 8 e-tiles
NT = S // 128    # 16 s/k-tiles
QB = 512
NQB = S // QB    # 4

_NC = None

# ---- custom DVE exp: p(s) ~ exp(s/64) (deg-4, a0=1), then p^8 -------------
# minimax-with-a0=1 coefficients for exp(u) on u in [-0.8, 0.8], folded with
# the 1/64 argument scale (b_i = a_i / 64^i).
_A = (0.99930331, 0.49979974, 0.17207327, 0.04243063)
EXP_B1 = _A[0] / 64.0
EXP_B2 = _A[1] / 64.0 ** 2
EXP_B3 = _A[2] / 64.0 ** 3
EXP_B4 = _A[3] / 64.0 ** 4

# exp-tile engine assignment: scores psum is split into [128, 512] j-half
# tiles (1 PSUM bank each, bufs=2) so the j-halves double-buffer each other
# and the exp engines never gate the next scores matmul. Balance ACT vs DVE
# by throughput: A-stream on ACT; 10 of 16 B-stream t's on DVE.
B_DVE_T = frozenset((1, 2, 4, 5, 7, 8, 10, 11, 13, 14))
A_DVE_T = frozenset()


def _register_dve_op(name, spec, subdim=False):
    import concourse.dve_ops as dvo
    from concourse.dve_uop import DveOpSpec
    from concourse.dve_spec import lower
    from concourse.dve_spec import _has_src1 as has_src1

    for op in dvo.OPS:
        if op.name == name:
            return op
    opcode = dvo._CUSTOM_DVE_ROW_BASE + len(dvo.OPS)
    assert opcode < 0x20
    dvo._SUB_OPCODE_FOR_NAME[name] = opcode
    shas = {}
    for ver in ("v3", "v4"):
        tmp = DveOpSpec(name=name, opcode=opcode, uops=lower(spec, ver=ver),
                        rd1_en=has_src1(spec))
        shas[ver] = tmp.sha(ver)
    op = dvo.DveOp(name, spec, subdim=subdim, uops_sha=shas)
    dvo.OPS.append(op)
    dvo.CUSTOM_DVE_SPECS[name] = spec
    return op


def _make_exp_ops():
    from concourse.dve_spec import (
        Spec, Src0, C0, C1, C2, C3, One, sq, _spill_c3_to_src1,
    )

    u = Src0
    body = _spill_c3_to_src1(((((u * C0 + C1) * u + C2) * u + C3) * u) + One)

    def _ref_poly(in0, in1, s0, s1, imm2):
        return ((((in0 * s0 + s1) * in0 + imm2) * in0 + in1) * in0
                + np.float32(1.0)).astype(np.float32)

    poly = _register_dve_op("EXP_POLY4_ANT", Spec(body=body, reference=_ref_poly))

    def _ref_pow8(in0, in1, s0, s1, imm2):
        q = (in0 * in0).astype(np.float32)
        q = (q * q).astype(np.float32)
        return (q * q).astype(np.float32)

    pow8 = _register_dve_op("POW8_ANT",
                            Spec(body=sq(sq(sq(Src0))), reference=_ref_pow8))
    return poly, pow8


EXP_POLY4, POW8 = _make_exp_ops()


def _mm(ap):
    """View a dram input AP with the matmul dtype (bitcast only for f32r)."""
    return ap.bitcast(F32R) if MM == F32R else ap


def _build():
    nc = bacc.Bacc("TRN2", target_bir_lowering=False, debug=False)
    xq_t = nc.dram_tensor("xq_t", [EMB, S], IN_DT, kind="ExternalInput").ap()
    xk_t = nc.dram_tensor("xk_t", [EMB, S], IN_DT, kind="ExternalInput").ap()
    xv_t = nc.dram_tensor("xv_t", [EMB, S], IN_DT, kind="ExternalInput").ap()
    # weights pre-arranged on host to [128, ET*CHD] / [128, 2*EMB] so the
    # load is 128 contiguous 4KB descriptors instead of 1024 strided ones
    wq_t = nc.dram_tensor("wq_t", [128, ET * CHD], IN_DT,
                          kind="ExternalInput").ap()
    wk_t = nc.dram_tensor("wk_t", [128, ET * CHD], IN_DT,
                          kind="ExternalInput").ap()
    wv_t = nc.dram_tensor("wv_t", [128, ET * CHD], IN_DT,
                          kind="ExternalInput").ap()
    wo_t = nc.dram_tensor("wo_t", [128, 2 * EMB], IN_DT,
                          kind="ExternalInput").ap()
    # partial outputs leave in the matmul dtype: halves the output-DMA
    # drain at the kernel tail; the host gather accumulates in fp32
    y_dt = F32 if MM == F32R else MM
    y = nc.dram_tensor("y", [S, EMB], y_dt, kind="ExternalOutput").ap()

    with tile.TileContext(nc) as tc:
        with tc.tile_pool(name="const", bufs=1) as cpool, \
             tc.tile_pool(name="wqk", bufs=2) as wpool, \
             tc.tile_pool(name="big", bufs=1) as big, \
             tc.tile_pool(name="usb", bufs=4) as usb, \
             tc.tile_pool(name="xp", bufs=8) as xp, \
             tc.tile_pool(name="pt", bufs=6) as ptp, \
             tc.tile_pool(name="esc", bufs=3) as escp, \
             tc.tile_pool(name="yp", bufs=2) as ypool, \
             tc.tile_pool(name="rp", bufs=2) as rpool, \
             tc.tile_pool(name="rd", bufs=4, space="DRAM") as rdram:

            # ---- static weights (wo DMA deferred past phase A) ----
            wo_sb = cpool.tile([128, 2, EMB], MM, name="wo_sb")
            b1c = cpool.tile([128, 1], F32, name="b1c")
            nc.vector.memset(b1c[:], EXP_B1)

            qT = big.tile([128, 2, S], MM, name="qT")     # later reused as O^T
            kT = big.tile([128, 2, S], MM, name="kT")
            v_sb = big.tile([128, NT, HG * (HD + 1)], MM, name="v_sb")
            if MM == F32R:
                nc.vector.memset(v_sb[:].bitcast(F32), 1.0)
            else:
                nc.vector.memset(v_sb[:], 1.0)     # ones cols survive

            # ---- phase A: projections ----
            warm0 = cpool.tile([128, QB], MM, name="warm0")
            nc.vector.memset(warm0[:], 1.0)
            with tc.tile_pool(name="psA", bufs=8, space="PSUM") as psA:
                # Q^T and K^T: out[m, q] accumulated over e; m-halves use
                # 4 psum banks each so one half's copies overlap the other
                # half's matmuls
                for name, xdram, wdram, dst in (
                        ("q", xq_t, wq_t, qT), ("k", xk_t, wk_t, kT)):
                    w_sb = wpool.tile([128, ET, CHD], MM, tag="w",
                                      name=f"w{name}_sb")
                    nc.sync.dma_start(
                        w_sb[:].rearrange("pi po m -> pi (po m)"),
                        _mm(wdram))
                    xts = []
                    for e in range(ET):
                        x_t = xp.tile([128, S], MM, tag="x", name=f"x_{name}{e}")
                        nc.sync.dma_start(
                            x_t[:], _mm(xdram)[e * 128:(e + 1) * 128, :])
                        xts.append(x_t)
                    for m in range(2):
                        pss = [psA.tile([128, QB], F32, tag="ps",
                                        name=f"ps_{name}{m}{i}")
                               for i in range(NQB)]
                        for e in range(ET):
                            for qb in range(NQB):
                                nc.tensor.matmul(
                                    pss[qb][:],
                                    w_sb[:, e, m * 128:(m + 1) * 128],
                                    xts[e][:, qb * QB:(qb + 1) * QB],
                                    start=(e == 0), stop=(e == ET - 1))
                        for qb in range(NQB):
                            cp = nc.scalar.copy if qb % 2 else \
                                nc.vector.tensor_copy
                            cp(dst[:, m, qb * QB:(qb + 1) * QB],
                               pss[qb][:])

                # V inputs: DMAs issued here, but the V matmuls themselves
                # are interleaved into the first attention quarter below,
                # where the PE has slack while the exp engines chew scores.
                wv_sb = wpool.tile([128, ET, CHD], MM, tag="w", name="wv_sb")
                nc.sync.dma_start(
                    wv_sb[:].rearrange("pi po m -> pi (po m)"),
                    _mm(wv_t))
                xv_tiles = []
                for e in range(ET):
                    x_t = xp.tile([128, S], MM, tag="x", name=f"x_v{e}")
                    nc.sync.dma_start(
                        x_t[:], _mm(xv_t)[e * 128:(e + 1) * 128, :])
                    xv_tiles.append(x_t)
                # deferred weight loads ride behind the xv DMAs
                nc.sync.dma_start(
                    wo_sb[:].rearrange("p ct n -> p (ct n)"), _mm(wo_t))

            # ---- phase B: attention, head-PAIRS packed on PE ----
            # Heads 2mh (rows 0-63) and 2mh+1 (rows 64-127) issue scores
            # matmuls into different PE row-groups + different psum banks, so
            # they run concurrently. q runs in QUARTERS (SH=512) so the U
            # accumulators take one PSUM bank each, leaving room for
            # double-buffered score tiles AND the interleaved V projection.
            u_list = [None] * HG
            NQH = 4
            SH = S // NQH
            with tc.tile_pool(name="psS", bufs=2, space="PSUM") as psS, \
                 tc.tile_pool(name="psU", bufs=1, space="PSUM") as psU:
                PV_LAG = 4
                pend = []
                for mh in range(2):
                    hA, hB = 2 * mh, 2 * mh + 1
                    uA = usb.tile([HD + 1, S], F32, tag="u", name=f"u{hA}")
                    uB = usb.tile([HD + 1, S], F32, tag="u", name=f"u{hB}")
                    u_list[hA], u_list[hB] = uA, uB
                    for qh in range(NQH):
                        qo = qh * SH
                        first_g = (mh == 0 and qh == 0)
                        uaccA = psU.tile([HD + 1, SH], F32, tag="uaccA",
                                         name=f"uaccA{mh}_{qh}")
                        uaccB = psU.tile([HD + 1, SH], F32, tag="uaccB",
                                         name=f"uaccB{mh}_{qh}")

                        # deferred work rides a flat pipeline ACROSS groups:
                        # each PV pair (and the group finish: uacc copies +
                        # normalization) is emitted PV_LAG iterations after
                        # its exp, deep into the next group's score stream,
                        # so the in-order PE queue never drains at group
                        # boundaries waiting for tail exps.
                        def mk_pv(t, pA, pB, uaccA=uaccA, uaccB=uaccB,
                                  hA=hA, hB=hB):
                            def f():
                                for h2, uacc, p_t in ((hA, uaccA, pA),
                                                      (hB, uaccB, pB)):
                                    nc.tensor.matmul(
                                        uacc[:],
                                        v_sb[:, t, h2 * (HD + 1):
                                             (h2 + 1) * (HD + 1)],
                                        p_t[:],
                                        start=(t == 0), stop=(t == NT - 1))
                            return f

                        def mk_fin(mh=mh, qh=qh, qo=qo, uA=uA, uB=uB,
                                   uaccA=uaccA, uaccB=uaccB, hA=hA, hB=hB):
                            def fin():
                                nc.vector.tensor_copy(uA[:, qo:qo + SH],
                                                      uaccA[:])
                                nc.scalar.copy(uB[:, qo:qo + SH], uaccB[:])
                                # softmax normalization for this quarter:
                                # r = 1/sums (row 64), broadcast r across 64
                                # partitions via DRAM-bounce DMA, O^T = U*r
                                # overwrites qT. All quarters but the last
                                # run on Pool hidden under later B work.
                                rp2 = rpool.tile([2, SH], F32, tag="rh",
                                                 name=f"rp2_{mh}{qh}")
                                nc.sync.dma_start(rp2[0:1, :],
                                                  uA[HD:HD + 1, qo:qo + SH])
                                nc.sync.dma_start(rp2[1:2, :],
                                                  uB[HD:HD + 1, qo:qo + SH])
                                r2 = rpool.tile([2, SH], F32, tag="rh2",
                                                name=f"r2_{mh}{qh}")
                                nc.vector.reciprocal_approx_fast(
                                    out=r2[:], in_=rp2[:])
                                last = (mh == 1 and qh == NQH - 1)
                                for h2, u_h in ((hA, uA), (hB, uB)):
                                    bp2 = 64 * (h2 % 2)
                                    rd = rdram.tile([1, SH], F32,
                                                    name=f"rd{h2}_{qh}")
                                    nc.sync.dma_start(
                                        rd[:], r2[h2 % 2:h2 % 2 + 1, :])
                                    rb = rpool.tile([HD, SH], F32, tag="rb",
                                                    name=f"rb{h2}_{qh}",
                                                    bufs=3)
                                    nc.sync.dma_start(
                                        rb[:], rd[:].to_broadcast([HD, SH]))
                                    eng = nc.vector if last else nc.gpsimd
                                    eng.tensor_tensor(
                                        qT[bp2:bp2 + HD, mh, qo:qo + SH],
                                        u_h[0:HD, qo:qo + SH],
                                        rb[:], MULT)
                            return fin

                        for t in range(NT):
                            if first_g:
                                # V projection s-tile t rides in the PE slack
                                v_ps = psS.tile([128, CHD], F32, tag="vps",
                                                name=f"ps_v{t}", bufs=1)
                                for e in range(ET):
                                    nc.tensor.matmul(
                                        v_ps[:],
                                        xv_tiles[e][:, t * 128:(t + 1) * 128],
                                        wv_sb[:, e, :],
                                        start=(e == 0), stop=(e == ET - 1))
                                vsrc = v_ps[:].rearrange("p (h d) -> p h d",
                                                         d=HD)
                                vdst = v_sb[:, t, :].rearrange(
                                    "p (h d) -> p h d",
                                    d=HD + 1)[:, :, 0:HD]
                                vcp = nc.scalar.copy if t % 2 else \
                                    nc.vector.tensor_copy
                                vcp(vdst, vsrc)
                            sps = []
                            for bp, st in ((0, "A"), (64, "B")):
                                # B-stream feeds the DVE 2-op chain (longer
                                # latency) -> give it a third buffer
                                sp = psS.tile([128, SH], F32,
                                              tag=f"sps{st}",
                                              name=f"sps{st}{mh}{qh}{t}",
                                              bufs=(3 if st == "B" else 2))
                                nc.tensor.matmul(
                                    sp[:],
                                    kT[bp:bp + HD, mh,
                                       t * 128:(t + 1) * 128],
                                    qT[bp:bp + HD, mh, qo:qo + SH],
                                    start=True, stop=True)
                                sps.append(sp)
                            pA = ptp.tile([128, SH], MM, tag="ptA",
                                          name=f"ptA{mh}{qh}{t}")
                            pB = ptp.tile([128, SH], MM, tag="ptB",
                                          name=f"ptB{mh}{qh}{t}")
                            # the first group is PE-paced (V rides in it),
                            # so DVE has idle latency budget there: run its
                            # whole B-stream on DVE to unload ACT, the
                            # overall pole engine
                            for sp, p_t, dve in (
                                    (sps[0], pA, t in A_DVE_T),
                                    (sps[1], pB, first_g or t in B_DVE_T)):
                                if dve:
                                    sc = escp.tile(
                                        [128, SH], F32, tag="esc",
                                        name=f"esc{mh}{qh}{t}")
                                    nc.vector._custom_dve(
                                        EXP_POLY4, out=sc[:], in0=sp[:],
                                        in1=b1c[:], s0=EXP_B4, s1=EXP_B3,
                                        imm2=EXP_B2)
                                    nc.vector._custom_dve(
                                        POW8, out=p_t[:], in0=sc[:])
                                else:
                                    nc.scalar.activation(
                                        p_t[:], sp[:], EXP,
                                        scale=0.125)
                            pend.append(mk_pv(t, pA, pB))
                            if t == NT - 1:
                                pend.append(mk_fin())
                            while len(pend) > PV_LAG:
                                pend.pop(0)()
                while pend:
                    pend.pop(0)()

            # ---- phase D: output projection (qT now holds O^T) ----
            with tc.tile_pool(name="psY", bufs=4, space="PSUM") as psY:
                # keep PE busy across the tail of the normalization chain
                tr1 = psY.tile([128, EMB], F32, tag="yps", name="warm1ps")
                for w in range(8):
                    nc.tensor.matmul(tr1[:, 0:QB], warm0[:, 0:128], warm0[:],
                                     start=True, stop=True)
                for s in range(NT):
                    y_ps = psY.tile([128, EMB], F32, tag="yps", name=f"yps{s}")
                    # ct-outer: each stationary serves both nb matmuls, so
                    # only two LDWEIGHTS per s-tile instead of four
                    for ct in range(2):
                        for nb in range(2):
                            nc.tensor.matmul(
                                y_ps[:, nb * QB:(nb + 1) * QB],
                                qT[:, ct, s * 128:(s + 1) * 128],
                                wo_sb[:, ct, nb * QB:(nb + 1) * QB],
                                start=(ct == 0), stop=(ct == 1))
                    y_sb = ypool.tile([128, EMB], y_dt, tag="ysb",
                                      name=f"ysb{s}")
                    cp = nc.scalar.copy if s % 2 else nc.vector.tensor_copy
                    cp(y_sb[:], y_ps[:])
                    nc.sync.dma_start(y[s * 128:(s + 1) * 128, :], y_sb[:])

    nc.compile()
    return nc


def get_nc():
    global _NC
    if _NC is None:
        _NC = _build()
    return _NC


def make_in_maps(query, key, value, Wq, Wk, Wv, Wo):
    import ml_dtypes
    np_dt = {F32R: np.float32, BF16: ml_dtypes.bfloat16,
             FP16: np.float16}[MM]
    query = np.asarray(query, dtype=np.float32)
    key = np.asarray(key, dtype=np.float32)
    value = np.asarray(value, dtype=np.float32)
    Wq = np.asarray(Wq, dtype=np.float32)
    Wk = np.asarray(Wk, dtype=np.float32)
    Wv = np.asarray(Wv, dtype=np.float32)
    Wo = np.asarray(Wo, dtype=np.float32)
    xt = {(n, b): np.ascontiguousarray(x[b].T).astype(np_dt)
          for n, x in (("q", query), ("k", key), ("v", value))
          for b in range(B)}

    def warr(wt):
        # [G*128, N] -> [128, G*N]: partition-contiguous so the device
        # load is 128 contiguous descriptors
        a = wt.reshape(-1, 128, wt.shape[1]).transpose(1, 0, 2)
        return np.ascontiguousarray(a.reshape(128, -1)).astype(np_dt)

    in_maps = []
    for c in range(8):
        b, g = divmod(c, 4)
        hs = slice(g * CHD, (g + 1) * CHD)
        in_maps.append({
            "xq_t": xt[("q", b)],
            "xk_t": xt[("k", b)],
            "xv_t": xt[("v", b)],
            "wq_t": warr(Wq[hs, :].T),
            "wk_t": warr(Wk[hs, :].T),
            "wv_t": warr(Wv[hs, :].T),
            "wo_t": warr(Wo[:, hs].T),
        })
    return in_maps


def gather(results):
    out = np.zeros((B, S, EMB), dtype=np.float32)
    for c in range(8):
        out[c // 4] += results[c]["y"].astype(np.float32)
    return out


def kernel(**inputs) -> np.ndarray:
    nc = get_nc()
    in_maps = make_in_maps(**inputs)
    res = run_bass_kernel_spmd(nc, in_maps, core_ids=list(range(8)))
    return gather(res.results)


# revision 59
# speedup vs baseline: 1.2298x; 1.2298x over previous
"""Multi-head attention (B=2, S=2048, EMB=1024, H=16, hd=64) on 8 TRN2 cores.

Sharding: core c -> batch b = c//4, head-group g = c%4 (4 heads, 256 emb dims).
Per core (all matmuls bf16: full-rate 1 cyc/row streaming):
  A) Q^T = Wq_g @ x_b^T   [256, 2048]   (transposed layout, contraction on emb)
     K^T = Wk_g @ x_b^T   [256, 2048]
     V   = x_b @ Wv_g^T   [2048, 256]   (natural layout, +ones column per head)
  B) per head h: S^T[k,q] = K_h @ Q_h^T (16 k-tiles x [128, 2048] psum)
     P^T = exp(S^T/8): split between ACT (native Exp) and DVE (custom 2-op
     chain: deg-4 poly p~exp(s/64) then p^8) so the softmax isn't ACT-bound;
     U_aug[65, 2048] += [V_h|1].T @ P^T  (row 64 = softmax sums)
  C) r = 1/sums (DVE recip approx); broadcast r over 64 partitions via
     indicator matmul; O^T = U^T * r on GpSimd (written over the Q^T buffer)
  D) y = O @ Wo_g^T partial [2048, 1024]; host sums the 4 head-group partials.
"""
import numpy as np

import concourse.bass as bass
import concourse.tile as tile
from concourse import bacc, mybir
from concourse.bass_utils import run_bass_kernel_spmd

import os

F32 = mybir.dt.float32
F32R = mybir.dt.float32r
BF16 = mybir.dt.bfloat16
FP16 = mybir.dt.float16
# matmul dtype: bf16 (1 cyc/row, ~6e-3) | fp16 (2 cyc/row, ~1e-3) | f32r
MM_DT_NAME = os.environ.get("MM_DT", "bf16")
MM = {"f32r": F32R, "bf16": BF16, "fp16": FP16}[MM_DT_NAME]
IN_DT = {"f32r": F32, "bf16": BF16, "fp16": FP16}[MM_DT_NAME]
EXP = mybir.ActivationFunctionType.Exp
MULT = mybir.AluOpType.mult

EMB = 1024
S = 2048
B = 2
HG = 4           # heads per core
HD = 64
CHD = HG * HD    # 256 emb dims per core
ET = EMB // 128  # 8 e-tiles
NT = S // 128    # 16 s/k-tiles
QB = 512
NQB = S // QB    # 4

_NC = None

# ---- custom DVE exp: p(s) ~ exp(s/64) (deg-4, a0=1), then p^8 -------------
# minimax-with-a0=1 coefficients for exp(u) on u in [-0.8, 0.8], folded with
# the 1/64 argument scale (b_i = a_i / 64^i).
_A = (0.99930331, 0.49979974, 0.17207327, 0.04243063)
EXP_B1 = _A[0] / 64.0
EXP_B2 = _A[1] / 64.0 ** 2
EXP_B3 = _A[2] / 64.0 ** 3
EXP_B4 = _A[3] / 64.0 ** 4

# exp-tile engine assignment: scores psum is split into [128, 512] j-half
# tiles (1 PSUM bank each, bufs=2) so the j-halves double-buffer each other
# and the exp engines never gate the next scores matmul. Balance ACT vs DVE
# by throughput: A-stream on ACT; 10 of 16 B-stream t's on DVE.
B_DVE_T = frozenset((1, 2, 4, 5, 7, 8, 10, 11, 13, 14))
A_DVE_T = frozenset()


def _register_dve_op(name, spec, subdim=False):
    import concourse.dve_ops as dvo
    from concourse.dve_uop import DveOpSpec
    from concourse.dve_spec import lower
    from concourse.dve_spec import _has_src1 as has_src1

    for op in dvo.OPS:
        if op.name == name:
            return op
    opcode = dvo._CUSTOM_DVE_ROW_BASE + len(dvo.OPS)
    assert opcode < 0x20
    dvo._SUB_OPCODE_FOR_NAME[name] = opcode
    shas = {}
    for ver in ("v3", "v4"):
        tmp = DveOpSpec(name=name, opcode=opcode, uops=lower(spec, ver=ver),
                        rd1_en=has_src1(spec))
        shas[ver] = tmp.sha(ver)
    op = dvo.DveOp(name, spec, subdim=subdim, uops_sha=shas)
    dvo.OPS.append(op)
    dvo.CUSTOM_DVE_SPECS[name] = spec
    return op


def _make_exp_ops():
    from concourse.dve_spec import (
        Spec, Src0, C0, C1, C2, C3, One, sq, _spill_c3_to_src1,
    )

    u = Src0
    body = _spill_c3_to_src1(((((u * C0 + C1) * u + C2) * u + C3) * u) + One)

    def _ref_poly(in0, in1, s0, s1, imm2):
        return ((((in0 * s0 + s1) * in0 + imm2) * in0 + in1) * in0
                + np.float32(1.0)).astype(np.float32)

    poly = _register_dve_op("EXP_POLY4_ANT", Spec(body=body, reference=_ref_poly))

    def _ref_pow8(in0, in1, s0, s1, imm2):
        q = (in0 * in0).astype(np.float32)
        q = (q * q).astype(np.float32)
        return (q * q).astype(np.float32)

    pow8 = _register_dve_op("POW8_ANT",
                            Spec(body=sq(sq(sq(Src0))), reference=_ref_pow8))
    return poly, pow8


EXP_POLY4, POW8 = _make_exp_ops()


def _mm(ap):
    """View a dram input AP with the matmul dtype (bitcast only for f32r)."""
    return ap.bitcast(F32R) if MM == F32R else ap


def _build():
    nc = bacc.Bacc("TRN2", target_bir_lowering=False, debug=False)
    xq_t = nc.dram_tensor("xq_t", [EMB, S], IN_DT, kind="ExternalInput").ap()
    xk_t = nc.dram_tensor("xk_t", [EMB, S], IN_DT, kind="ExternalInput").ap()
    xv_t = nc.dram_tensor("xv_t", [EMB, S], IN_DT, kind="ExternalInput").ap()
    # weights pre-arranged on host to [128, ET*CHD] / [128, 2*EMB] so the
    # load is 128 contiguous 4KB descriptors instead of 1024 strided ones
    wq_t = nc.dram_tensor("wq_t", [128, ET * CHD], IN_DT,
                          kind="ExternalInput").ap()
    wk_t = nc.dram_tensor("wk_t", [128, ET * CHD], IN_DT,
                          kind="ExternalInput").ap()
    wv_t = nc.dram_tensor("wv_t", [128, ET * CHD], IN_DT,
                          kind="ExternalInput").ap()
    wo_t = nc.dram_tensor("wo_t", [128, 2 * EMB], IN_DT,
                          kind="ExternalInput").ap()
    # partial outputs leave in the matmul dtype: halves the output-DMA
    # drain at the kernel tail; the host gather accumulates in fp32
    y_dt = F32 if MM == F32R else MM
    y = nc.dram_tensor("y", [S, EMB], y_dt, kind="ExternalOutput").ap()

    with tile.TileContext(nc) as tc:
        with tc.tile_pool(name="const", bufs=1) as cpool, \
             tc.tile_pool(name="wqk", bufs=2) as wpool, \
             tc.tile_pool(name="big", bufs=1) as big, \
             tc.tile_pool(name="usb", bufs=4) as usb, \
             tc.tile_pool(name="xp", bufs=8) as xp, \
             tc.tile_pool(name="pt", bufs=6) as ptp, \
             tc.tile_pool(name="esc", bufs=3) as escp, \
             tc.tile_pool(name="yp", bufs=2) as ypool, \
             tc.tile_pool(name="rp", bufs=2) as rpool, \
             tc.tile_pool(name="rd", bufs=4, space="DRAM") as rdram:

            # ---- static weights (wo DMA deferred past phase A) ----
            wo_sb = cpool.tile([128, 2, EMB], MM, name="wo_sb")
            b1c = cpool.tile([128, 1], F32, name="b1c")
            nc.vector.memset(b1c[:], EXP_B1)

            qT = big.tile([128, 2, S], MM, name="qT")     # later reused as O^T
            kT = big.tile([128, 2, S], MM, name="kT")
            v_sb = big.tile([128, NT, HG * (HD + 1)], MM, name="v_sb")
            if MM == F32R:
                nc.vector.memset(v_sb[:].bitcast(F32), 1.0)
            else:
                nc.vector.memset(v_sb[:], 1.0)     # ones cols survive

            # ---- phase A: projections ----
            warm0 = cpool.tile([128, QB], MM, name="warm0")
            nc.vector.memset(warm0[:], 1.0)
            with tc.tile_pool(name="psA", bufs=8, space="PSUM") as psA:
                # Q^T and K^T: out[m, q] accumulated over e; m-halves use
                # 4 psum banks each so one half's copies overlap the other
                # half's matmuls
                for name, xdram, wdram, dst in (
                        ("q", xq_t, wq_t, qT), ("k", xk_t, wk_t, kT)):
                    w_sb = wpool.tile([128, ET, CHD], MM, tag="w",
                                      name=f"w{name}_sb")
                    nc.sync.dma_start(
                        w_sb[:].rearrange("pi po m -> pi (po m)"),
                        _mm(wdram))
                    xts = []
                    for e in range(ET):
                        x_t = xp.tile([128, S], MM, tag="x", name=f"x_{name}{e}")
                        nc.sync.dma_start(
                            x_t[:], _mm(xdram)[e * 128:(e + 1) * 128, :])
                        xts.append(x_t)
                    for m in range(2):
                        pss = [psA.tile([128, QB], F32, tag="ps",
                                        name=f"ps_{name}{m}{i}")
                               for i in range(NQB)]
                        for e in range(ET):
                            for qb in range(NQB):
                                nc.tensor.matmul(
                                    pss[qb][:],
                                    w_sb[:, e, m * 128:(m + 1) * 128],
                                    xts[e][:, qb * QB:(qb + 1) * QB],
                                    start=(e == 0), stop=(e == ET - 1))
                        for qb in range(NQB):
                            cp = nc.scalar.copy if qb % 2 else \
                                nc.vector.tensor_copy
                            cp(dst[:, m, qb * QB:(qb + 1) * QB],
                               pss[qb][:])

                # V inputs: DMAs issued here, but the V matmuls themselves
                # are interleaved into the first attention quarter below,
                # where the PE has slack while the exp engines chew scores.
                wv_sb = wpool.tile([128, ET, CHD], MM, tag="w", name="wv_sb")
                nc.sync.dma_start(
                    wv_sb[:].rearrange("pi po m -> pi (po m)"),
                    _mm(wv_t))
                # load xv in column-quarter order across ALL e-tiles: the
                # V matmuls consume xv column-slices t-by-t but need every
                # e-tile for each t, so landing the low columns of all
                # e-tiles first lets V(0) start ~3us earlier — otherwise
                # the PE idles on the xv DMA tail entering phase B and the
                # HAM clock gate re-throttles it to half rate for ~14us.
                xv_tiles = [xp.tile([128, S], MM, tag="x", name=f"x_v{e}")
                            for e in range(ET)]
                for q4 in range(4):
                    cs = slice(q4 * (S // 4), (q4 + 1) * (S // 4))
                    for e in range(ET):
                        nc.sync.dma_start(
                            xv_tiles[e][:, cs],
                            _mm(xv_t)[e * 128:(e + 1) * 128, cs])
                # deferred weight loads ride behind the xv DMAs
                nc.sync.dma_start(
                    wo_sb[:].rearrange("p ct n -> p (ct n)"), _mm(wo_t))

            # ---- phase B: attention, head-PAIRS packed on PE ----
            # Heads 2mh (rows 0-63) and 2mh+1 (rows 64-127) issue scores
            # matmuls into different PE row-groups + different psum banks, so
            # they run concurrently. q runs in QUARTERS (SH=512) so the U
            # accumulators take one PSUM bank each, leaving room for
            # double-buffered score tiles AND the interleaved V projection.
            u_list = [None] * HG
            NQH = 4
            SH = S // NQH
            with tc.tile_pool(name="psS", bufs=2, space="PSUM") as psS, \
                 tc.tile_pool(name="psU", bufs=1, space="PSUM") as psU:
                PV_LAG = 4
                pend = []
                for mh in range(2):
                    hA, hB = 2 * mh, 2 * mh + 1
                    uA = usb.tile([HD + 1, S], F32, tag="u", name=f"u{hA}")
                    uB = usb.tile([HD + 1, S], F32, tag="u", name=f"u{hB}")
                    u_list[hA], u_list[hB] = uA, uB
                    for qh in range(NQH):
                        qo = qh * SH
                        first_g = (mh == 0 and qh == 0)
                        uaccA = psU.tile([HD + 1, SH], F32, tag="uaccA",
                                         name=f"uaccA{mh}_{qh}")
                        uaccB = psU.tile([HD + 1, SH], F32, tag="uaccB",
                                         name=f"uaccB{mh}_{qh}")

                        # deferred work rides a flat pipeline ACROSS groups:
                        # each PV pair (and the group finish: uacc copies +
                        # normalization) is emitted PV_LAG iterations after
                        # its exp, deep into the next group's score stream,
                        # so the in-order PE queue never drains at group
                        # boundaries waiting for tail exps.
                        def mk_pv(t, pA, pB, uaccA=uaccA, uaccB=uaccB,
                                  hA=hA, hB=hB):
                            def f():
                                for h2, uacc, p_t in ((hA, uaccA, pA),
                                                      (hB, uaccB, pB)):
                                    nc.tensor.matmul(
                                        uacc[:],
                                        v_sb[:, t, h2 * (HD + 1):
                                             (h2 + 1) * (HD + 1)],
                                        p_t[:],
                                        start=(t == 0), stop=(t == NT - 1))
                            return f

                        def mk_fin(mh=mh, qh=qh, qo=qo, uA=uA, uB=uB,
                                   uaccA=uaccA, uaccB=uaccB, hA=hA, hB=hB):
                            def fin():
                                nc.vector.tensor_copy(uA[:, qo:qo + SH],
                                                      uaccA[:])
                                nc.scalar.copy(uB[:, qo:qo + SH], uaccB[:])
                                # softmax normalization for this quarter:
                                # r = 1/sums (row 64), broadcast r across 64
                                # partitions via DRAM-bounce DMA, O^T = U*r
                                # overwrites qT. All quarters but the last
                                # run on Pool hidden under later B work.
                                rp2 = rpool.tile([2, SH], F32, tag="rh",
                                                 name=f"rp2_{mh}{qh}")
                                nc.sync.dma_start(rp2[0:1, :],
                                                  uA[HD:HD + 1, qo:qo + SH])
                                nc.sync.dma_start(rp2[1:2, :],
                                                  uB[HD:HD + 1, qo:qo + SH])
                                r2 = rpool.tile([2, SH], F32, tag="rh2",
                                                name=f"r2_{mh}{qh}")
                                nc.vector.reciprocal_approx_fast(
                                    out=r2[:], in_=rp2[:])
                                last = (mh == 1 and qh == NQH - 1)
                                for h2, u_h in ((hA, uA), (hB, uB)):
                                    bp2 = 64 * (h2 % 2)
                                    rd = rdram.tile([1, SH], F32,
                                                    name=f"rd{h2}_{qh}")
                                    nc.sync.dma_start(
                                        rd[:], r2[h2 % 2:h2 % 2 + 1, :])
                                    rb = rpool.tile([HD, SH], F32, tag="rb",
                                                    name=f"rb{h2}_{qh}",
                                                    bufs=3)
                                    nc.sync.dma_start(
                                        rb[:], rd[:].to_broadcast([HD, SH]))
                                    eng = nc.vector if last else nc.gpsimd
                                    eng.tensor_tensor(
                                        qT[bp2:bp2 + HD, mh, qo:qo + SH],
                                        u_h[0:HD, qo:qo + SH],
                                        rb[:], MULT)
                            return fin

                        for t in range(NT):
                            if first_g:
                                # V projection s-tile t rides in the PE slack
                                v_ps = psS.tile([128, CHD], F32, tag="vps",
                                                name=f"ps_v{t}", bufs=1)
                                for e in range(ET):
                                    nc.tensor.matmul(
                                        v_ps[:],
                                        xv_tiles[e][:, t * 128:(t + 1) * 128],
                                        wv_sb[:, e, :],
                                        start=(e == 0), stop=(e == ET - 1))
                                vsrc = v_ps[:].rearrange("p (h d) -> p h d",
                                                         d=HD)
                                vdst = v_sb[:, t, :].rearrange(
                                    "p (h d) -> p h d",
                                    d=HD + 1)[:, :, 0:HD]
                                vcp = nc.scalar.copy if t % 2 else \
                                    nc.vector.tensor_copy
                                vcp(vdst, vsrc)
                            sps = []
                            for bp, st in ((0, "A"), (64, "B")):
                                # B-stream feeds the DVE 2-op chain (longer
                                # latency) -> give it a third buffer
                                sp = psS.tile([128, SH], F32,
                                              tag=f"sps{st}",
                                              name=f"sps{st}{mh}{qh}{t}",
                                              bufs=(3 if st == "B" else 2))
                                nc.tensor.matmul(
                                    sp[:],
                                    kT[bp:bp + HD, mh,
                                       t * 128:(t + 1) * 128],
                                    qT[bp:bp + HD, mh, qo:qo + SH],
                                    start=True, stop=True)
                                sps.append(sp)
                            pA = ptp.tile([128, SH], MM, tag="ptA",
                                          name=f"ptA{mh}{qh}{t}")
                            pB = ptp.tile([128, SH], MM, tag="ptB",
                                          name=f"ptB{mh}{qh}{t}")
                            for sp, p_t, dve in (
                                    (sps[0], pA, t in A_DVE_T),
                                    (sps[1], pB, t in B_DVE_T)):
                                if dve:
                                    sc = escp.tile(
                                        [128, SH], F32, tag="esc",
                                        name=f"esc{mh}{qh}{t}")
                                    nc.vector._custom_dve(
                                        EXP_POLY4, out=sc[:], in0=sp[:],
                                        in1=b1c[:], s0=EXP_B4, s1=EXP_B3,
                                        imm2=EXP_B2)
                                    nc.vector._custom_dve(
                                        POW8, out=p_t[:], in0=sc[:])
                                else:
                                    nc.scalar.activation(
                                        p_t[:], sp[:], EXP,
                                        scale=0.125)
                            pend.append(mk_pv(t, pA, pB))
                            if t == NT - 1:
                                pend.append(mk_fin())
                            while len(pend) > PV_LAG:
                                pend.pop(0)()
                while pend:
                    pend.pop(0)()

            # ---- phase D: output projection (qT now holds O^T) ----
            with tc.tile_pool(name="psY", bufs=4, space="PSUM") as psY:
                # keep PE busy across the tail of the normalization chain
                tr1 = psY.tile([128, EMB], F32, tag="yps", name="warm1ps")
                for w in range(8):
                    nc.tensor.matmul(tr1[:, 0:QB], warm0[:, 0:128], warm0[:],
                                     start=True, stop=True)
                for s in range(NT):
                    y_ps = psY.tile([128, EMB], F32, tag="yps", name=f"yps{s}")
                    for nb in range(2):
                        for ct in range(2):
                            nc.tensor.matmul(
                                y_ps[:, nb * QB:(nb + 1) * QB],
                                qT[:, ct, s * 128:(s + 1) * 128],
                                wo_sb[:, ct, nb * QB:(nb + 1) * QB],
                                start=(ct == 0), stop=(ct == 1))
                    y_sb = ypool.tile([128, EMB], y_dt, tag="ysb",
                                      name=f"ysb{s}")
                    cp = nc.scalar.copy if s % 2 else nc.vector.tensor_copy
                    cp(y_sb[:], y_ps[:])
                    nc.sync.dma_start(y[s * 128:(s + 1) * 128, :], y_sb[:])

    nc.compile()
    return nc


def get_nc():
    global _NC
    if _NC is None:
        _NC = _build()
    return _NC


def make_in_maps(query, key, value, Wq, Wk, Wv, Wo):
    import ml_dtypes
    np_dt = {F32R: np.float32, BF16: ml_dtypes.bfloat16,
             FP16: np.float16}[MM]
    query = np.asarray(query, dtype=np.float32)
    key = np.asarray(key, dtype=np.float32)
    value = np.asarray(value, dtype=np.float32)
    Wq = np.asarray(Wq, dtype=np.float32)
    Wk = np.asarray(Wk, dtype=np.float32)
    Wv = np.asarray(Wv, dtype=np.float32)
    Wo = np.asarray(Wo, dtype=np.float32)
    xt = {(n, b): np.ascontiguousarray(x[b].T).astype(np_dt)
          for n, x in (("q", query), ("k", key), ("v", value))
          for b in range(B)}

    def warr(wt):
        # [G*128, N] -> [128, G*N]: partition-contiguous so the device
        # load is 128 contiguous descriptors
        a = wt.reshape(-1, 128, wt.shape[1]).transpose(1, 0, 2)
        return np.ascontiguousarray(a.reshape(128, -1)).astype(np_dt)

    in_maps = []
    for c in range(8):
        b, g = divmod(c, 4)
        hs = slice(g * CHD, (g + 1) * CHD)
        in_maps.append({
            "xq_t": xt[("q", b)],
            "xk_t": xt[("k", b)],
            "xv_t": xt[("v", b)],
            "wq_t": warr(Wq[hs, :].T),
            "wk_t": warr(Wk[hs, :].T),
            "wv_t": warr(Wv[hs, :].T),
            "wo_t": warr(Wo[:, hs].T),
        })
    return in_maps


def gather(results):
    out = np.zeros((B, S, EMB), dtype=np.float32)
    for c in range(8):
        out[c // 4] += results[c]["y"].astype(np.float32)
    return out


def kernel(**inputs) -> np.ndarray:
    nc = get_nc()
    in_maps = make_in_maps(**inputs)
    res = run_bass_kernel_spmd(nc, in_maps, core_ids=list(range(8)))
    return gather(res.results)
